# revision 47
# baseline (speedup 1.0000x reference)
"""Trainium2 Bass kernel for nn_CausalSelfAttention (sparse_attention).

Computes, for embedding [S=8192, H=128] and per-row copy scalars c:
    q = emb @ Wq.T ; k = emb @ Wk.T ; v = emb @ Wv.T
    scores = q @ k.T / sqrt(H)
    sm     = softmax(scores, axis=-1)
    alphas = (sm + c[:, None]) / l1norm_row(sm + c)
    context = alphas @ v
Returns (context [S, H], alphas [S, S]) as float32, matching the reference.

Key algebra: since softmax rows sum to 1 and c >= 0,
    alphas = a * E + b         with E = exp(scores/sqrt(H)),
                                    a = 1 / (D * (1 + S*c)),  D = rowsum(E),
                                    b = c / (1 + S*c)
    context = a * (E @ v) + b * colsum(v)

Sharding: rows are split across 8 NeuronCores (1024 rows each); K/V are
replicated. The tiny projections (0.3% of FLOPs) run on the host in f64;
each core runs two interleaved exp chains over its [1024, 8192] score
block: a row-major one that feeds the (memory-bound) alphas writeback and
a transposed one that feeds the E^T @ V context contraction on the PE.
The interleave keeps ScalarE (the bottleneck engine) continuously busy.
"""

import sys

if "/opt/trn_rl_repo" not in sys.path:
    sys.path.insert(0, "/opt/trn_rl_repo")

from contextlib import ExitStack

import numpy as np

import concourse.bass as bass
import concourse.tile as tile
from concourse import bacc, mybir
from concourse.bass_utils import run_bass_kernel_spmd
from concourse.masks import make_identity

S = 8192
H = 128
M = 8  # cores
R = S // M  # rows per core = 1024
NT = R // 128  # row-tiles per core = 8
NC = S // 128  # key chunks = 64
SCALE = 1.0 / float(np.sqrt(H))

F16 = mybir.dt.float16
F32 = mybir.dt.float32
EXP = mybir.ActivationFunctionType.Exp
MULT = mybir.AluOpType.mult
ADD = mybir.AluOpType.add

_CACHED_NC = None


def build():
    nc = bacc.Bacc("TRN2", target_bir_lowering=False, debug=False)

    kt = nc.dram_tensor("kt", [H, S], F16, kind="ExternalInput").ap()
    qt = nc.dram_tensor("qt", [H, R], F16, kind="ExternalInput").ap()
    vc = nc.dram_tensor("vc", [128, S], F16, kind="ExternalInput").ap()
    copy_col = nc.dram_tensor("copy_col", [128, NT], F32, kind="ExternalInput").ap()
    sumv_row = nc.dram_tensor("sumv_row", [1, H], F32, kind="ExternalInput").ap()

    alphas = nc.dram_tensor("alphas", [R, S], F32, kind="ExternalOutput").ap()
    ctx_out = nc.dram_tensor("ctx", [R, H], F32, kind="ExternalOutput").ap()

    with tile.TileContext(nc) as tc, ExitStack() as ctx:
        # ---- persistent SBUF ----
        sb = ctx.enter_context(tc.tile_pool(name="sb", bufs=1))
        kt_sb = sb.tile([H, S], F16, name="kt_sb")  # K^T: [h, s']
        qt_sb = sb.tile([H, R], F16, name="qt_sb")  # Q^T: [h, r]
        v_sb = sb.tile([128, S], F16, name="v_sb")  # chunk c at cols c*H: [s', h]
        v_r = sb.tile([128, S], mybir.dt.float32r, name="v_r")  # rounded copy
        ccol_sb = sb.tile([128, NT], F32, name="ccol_sb")
        ident = sb.tile([128, 128], F32, name="ident")
        d_sb = sb.tile([128, NT], F32, name="d_sb")  # rowsum(E) per row-tile
        a_sb = sb.tile([128, NT], F32, name="a_sb")  # affine mult per row-tile
        b_sb = sb.tile([128, NT], F32, name="b_sb")  # affine add per row-tile
        sumv_sb = sb.tile([1, H], F32, name="sumv_sb")
        sumvb_sb = sb.tile([128, H], F32, name="sumvb_sb")
        ones1_sb = sb.tile([1, 128], F32, name="ones1_sb")

        # qt + first kt piece on the sync queue (gate the first matmuls);
        # everything else spread over other engine queues so issue slots
        # don't serialize
        nc.sync.dma_start(out=qt_sb[:, :], in_=qt[:, :])
        nc.sync.dma_start(out=kt_sb[:, 0 : S // 4], in_=kt[:, 0 : S // 4])
        for p in range(1, 4):
            cols = slice(p * (S // 4), (p + 1) * (S // 4))
            nc.scalar.dma_start(out=kt_sb[:, cols], in_=kt[:, cols])
        nc.gpsimd.dma_start(out=v_sb[:, :], in_=vc[:, :])
        nc.gpsimd.dma_start(out=ccol_sb[:, :], in_=copy_col[:, :])
        nc.gpsimd.dma_start(out=sumv_sb[0:1, :], in_=sumv_row[0:1, :])
        # round V to float32r for the full-rate context matmuls
        for p in range(4):
            cols = slice(p * (S // 4), (p + 1) * (S // 4))
            nc.vector.tensor_copy(v_r[:, cols], v_sb[:, cols])
        make_identity(nc, ident[:, :])

        ep = ctx.enter_context(tc.tile_pool(name="ep", bufs=2))
        ctxT_sb = ep.tile([128, R], F32, name="ctxT_sb", bufs=1)

        with tc.tile_pool(name="b_ps", bufs=1, space="PSUM") as b_ps, \
             tc.tile_pool(name="c_ps", bufs=1, space="PSUM") as c_ps, \
             tc.tile_pool(name="ctx_ps", bufs=1, space="PSUM") as ctx_ps, \
             tc.tile_pool(name="b_e", bufs=2) as b_e, \
             tc.tile_pool(name="c_et", bufs=3) as c_et, \
             tc.tile_pool(name="b_small", bufs=4) as b_small:
            ps_ctx = ctx_ps.tile([128, R], F32, name="ps_ctx")

            pending_ctx = []  # (chunk_idx, et) awaiting their ctx matmuls

            def flush_ctx():
                while pending_ctx:
                    c, et = pending_ctx.pop(0)
                    vslice = v_r[:, c * 128 : (c + 1) * 128]
                    for j in range(2):
                        nc.tensor.matmul(
                            ps_ctx[:, j * 512 : (j + 1) * 512],
                            vslice,
                            et[:, j * 512 : (j + 1) * 512],
                            start=(c == 0), stop=(c == NC - 1),
                            skip_group_check=True,
                        )

            def c_chunk(c):
                # transposed-score chunk: scores^T -> exp (float32r out); the
                # E^T @ V matmuls are deferred one chunk (flush_ctx) so the
                # PE never waits on this chunk's exp.
                ps_t = c_ps.tile([128, R], F32, tag="cscore", name=f"cs{c}")
                kslice = kt_sb[:, c * 128 : (c + 1) * 128]
                for j in range(2):
                    nc.tensor.matmul(
                        ps_t[:, j * 512 : (j + 1) * 512],
                        kslice,
                        qt_sb[:, j * 512 : (j + 1) * 512],
                        start=True, stop=True,
                    )
                flush_ctx()
                et = c_et.tile([128, R], mybir.dt.float32r, tag="Et", name=f"et{c}")
                nc.scalar.activation(et[:, :], ps_t[:, :], EXP, scale=SCALE)
                pending_ctx.append((c, et))

            # Interleaved main loop: per B score block (4 matmuls + one
            # FD=2048 exp), two-ish transposed chunks. Chunks are front-
            # loaded (9/9/9/9/9/9/6/4 per row-tile) so even the last
            # row-tile's exps stay covered without a bare C-C lockstep tail.
            chunks_per_tile = [9, 9, 9, 9, 9, 9, 6, 4]
            cchunk = 0
            for t in range(NT):
                budget = chunks_per_tile[t]
                e_t = b_e.tile([128, S], F32, tag="E", name=f"e{t}")
                dparts = b_small.tile([128, 4], F32, tag="dp", name=f"dp{t}")
                qslice = qt_sb[:, t * 128 : (t + 1) * 128]
                for g in range(4):
                    ps_s = b_ps.tile([128, 2048], F32, tag="bscore", name=f"bs{t}_{g}")
                    for j in range(4):
                        col = (g * 4 + j) * 512
                        nc.tensor.matmul(
                            ps_s[:, j * 512 : (j + 1) * 512],
                            qslice,
                            kt_sb[:, col : col + 512],
                            start=True, stop=True,
                        )
                    nc.scalar.activation(
                        e_t[:, g * 2048 : (g + 1) * 2048],
                        ps_s[:, :],
                        EXP,
                        scale=SCALE,
                        accum_out=dparts[:, g : g + 1],
                    )
                    take = min(budget, 3 if g == 3 else 2)
                    for _ in range(take):
                        c_chunk(cchunk)
                        cchunk += 1
                    budget -= take
                # per-row scalars: u = 1 + S*c ; a = 1/(D*u) ; b = c/u
                dcol = d_sb[:, t : t + 1]
                nc.vector.tensor_reduce(dcol, dparts[:, :], mybir.AxisListType.X, ADD)
                u_t = b_small.tile([128, 1], F32, tag="u", name=f"u{t}")
                ru_t = b_small.tile([128, 1], F32, tag="ru", name=f"ru{t}")
                du_t = b_small.tile([128, 1], F32, tag="du", name=f"du{t}")
                ccol = ccol_sb[:, t : t + 1]
                nc.vector.tensor_scalar(u_t[:, :], ccol, float(S), 1.0, MULT, ADD)
                nc.vector.reciprocal(ru_t[:, :], u_t[:, :])
                nc.vector.tensor_mul(du_t[:, :], dcol, u_t[:, :])
                nc.vector.reciprocal(a_sb[:, t : t + 1], du_t[:, :])
                nc.vector.tensor_mul(b_sb[:, t : t + 1], ccol, ru_t[:, :])
                # alphas tile = a*E + b, in place, then write back; split in
                # four so no single DVE op blocks the pipeline for long
                for g in range(4):
                    cols = slice(g * 2048, (g + 1) * 2048)
                    nc.vector.tensor_scalar(
                        e_t[:, cols], e_t[:, cols],
                        a_sb[:, t : t + 1], b_sb[:, t : t + 1],
                        MULT, ADD,
                    )
                nc.sync.dma_start(
                    out=alphas[t * 128 : (t + 1) * 128, :], in_=e_t[:, :]
                )

            while cchunk < NC:
                c_chunk(cchunk)
                cchunk += 1
            flush_ctx()
            nc.vector.tensor_copy(ctxT_sb[:, :], ps_ctx[:, :])

        # epilogue: transpose ctx^T back to row-major, scale by a, add b*colsum(v)
        with tc.tile_pool(name="ep_ps", bufs=2, space="PSUM") as ep_ps:
            # replicate colsum(v) across partitions via a K=1 fp32 matmul
            nc.vector.memset(ones1_sb[0:1, :], 1.0)
            psvb = ep_ps.tile([128, H], F32, tag="psvb", name="psvb", bufs=1)
            nc.tensor.matmul(
                psvb[:, :], ones1_sb[0:1, :], sumv_sb[0:1, :], start=True, stop=True
            )
            nc.vector.tensor_copy(sumvb_sb[:, :], psvb[:, :])
            for t in range(NT):
                ps_tr = ep_ps.tile([128, 128], F32, tag="pstr", name=f"ptr{t}")
                nc.tensor.matmul(
                    ps_tr[:, :],
                    ctxT_sb[:, t * 128 : (t + 1) * 128],
                    ident[:, :],
                    is_transpose=True, start=True, stop=True,
                )
                co = ep.tile([128, H], F32, tag="co", name=f"co{t}")
                nc.vector.tensor_scalar(
                    co[:, :], ps_tr[:, :], a_sb[:, t : t + 1], None, MULT
                )
                # co += b * colsum(v)  (sumvb is colsum(v) replicated per partition)
                nc.vector.scalar_tensor_tensor(
                    co[:, :], sumvb_sb[:, :], b_sb[:, t : t + 1], co[:, :],
                    MULT, ADD,
                )
                nc.sync.dma_start(
                    out=ctx_out[t * 128 : (t + 1) * 128, :], in_=co[:, :]
                )

    nc.compile()
    return nc


def kernel(embedding, copy, Wq, Wk, Wv, mask, _trace=False):
    global _CACHED_NC
    if _CACHED_NC is None:
        _CACHED_NC = build()
    nc = _CACHED_NC

    emb64 = np.asarray(embedding, dtype=np.float64)
    k_full = emb64 @ np.asarray(Wk, dtype=np.float64).T  # [S, H]
    q_full = emb64 @ np.asarray(Wq, dtype=np.float64).T
    v_full = emb64 @ np.asarray(Wv, dtype=np.float64).T
    kt_np = np.ascontiguousarray(k_full.T.astype(np.float16))  # [H, S]
    # v_sb layout: [p, c*H + h] = V[c*128 + p, h]
    vc_np = np.ascontiguousarray(
        v_full.reshape(NC, 128, H).transpose(1, 0, 2).reshape(128, S).astype(np.float16)
    )
    sumv_np = v_full.sum(axis=0).astype(np.float32)[None, :]

    in_maps = []
    for m in range(M):
        rows = slice(m * R, (m + 1) * R)
        c_m = np.asarray(copy[rows], dtype=np.float32).reshape(NT, 128)
        in_maps.append(
            {
                "kt": kt_np,
                "qt": np.ascontiguousarray(q_full[rows].T.astype(np.float16)),
                "vc": vc_np,
                "copy_col": np.ascontiguousarray(c_m.T),
                "sumv_row": sumv_np,
            }
        )

    res = run_bass_kernel_spmd(nc, in_maps, core_ids=list(range(M)), trace=_trace)
    global _LAST_RES
    _LAST_RES = res
    alphas = np.concatenate([r["alphas"] for r in res.results], axis=0)
    context = np.concatenate([r["ctx"] for r in res.results], axis=0)
    if _trace:
        return (context, alphas), res
    return context, alphas


# revision 51
# speedup vs baseline: 1.0117x; 1.0117x over previous
"""Trainium2 Bass kernel for nn_CausalSelfAttention (sparse_attention).

Computes, for embedding [S=8192, H=128] and per-row copy scalars c:
    q = emb @ Wq.T ; k = emb @ Wk.T ; v = emb @ Wv.T
    scores = q @ k.T / sqrt(H)
    sm     = softmax(scores, axis=-1)
    alphas = (sm + c[:, None]) / l1norm_row(sm + c)
    context = alphas @ v
Returns (context [S, H], alphas [S, S]) as float32, matching the reference.

Key algebra: since softmax rows sum to 1 and c >= 0,
    alphas = a * E + b         with E = exp(scores/sqrt(H)),
                                    a = 1 / (D * (1 + S*c)),  D = rowsum(E),
                                    b = c / (1 + S*c)
    context = a * (E @ v) + b * colsum(v)

Sharding: rows are split across 8 NeuronCores (1024 rows each); K/V are
replicated. The tiny projections (0.3% of FLOPs) run on the host in f64;
each core runs two interleaved exp chains over its [1024, 8192] score
block: a row-major one that feeds the (memory-bound) alphas writeback and
a transposed one that feeds the E^T @ V context contraction on the PE.
The interleave keeps ScalarE (the bottleneck engine) continuously busy.
"""

import sys

if "/opt/trn_rl_repo" not in sys.path:
    sys.path.insert(0, "/opt/trn_rl_repo")

from contextlib import ExitStack

import numpy as np

import concourse.bass as bass
import concourse.tile as tile
from concourse import bacc, mybir
from concourse.bass_utils import run_bass_kernel_spmd
from concourse.masks import make_identity

S = 8192
H = 128
M = 8  # cores
R = S // M  # rows per core = 1024
NT = R // 128  # row-tiles per core = 8
NC = S // 128  # key chunks = 64
SCALE = 1.0 / float(np.sqrt(H))

F16 = mybir.dt.float16
F32 = mybir.dt.float32
EXP = mybir.ActivationFunctionType.Exp
MULT = mybir.AluOpType.mult
ADD = mybir.AluOpType.add

_CACHED_NC = None


def build():
    nc = bacc.Bacc("TRN2", target_bir_lowering=False, debug=False)

    kt = nc.dram_tensor("kt", [H, S], F16, kind="ExternalInput").ap()
    qt = nc.dram_tensor("qt", [H, R], F16, kind="ExternalInput").ap()
    vc = nc.dram_tensor("vc", [128, S], F16, kind="ExternalInput").ap()
    copy_col = nc.dram_tensor("copy_col", [128, NT], F32, kind="ExternalInput").ap()
    sumv_row = nc.dram_tensor("sumv_row", [1, H], F32, kind="ExternalInput").ap()

    alphas = nc.dram_tensor("alphas", [R, S], F32, kind="ExternalOutput").ap()
    ctx_out = nc.dram_tensor("ctx", [R, H], F32, kind="ExternalOutput").ap()

    with tile.TileContext(nc) as tc, ExitStack() as ctx:
        # ---- persistent SBUF ----
        sb = ctx.enter_context(tc.tile_pool(name="sb", bufs=1))
        kt_sb = sb.tile([H, S], F16, name="kt_sb")  # K^T: [h, s']
        qt_sb = sb.tile([H, R], F16, name="qt_sb")  # Q^T: [h, r]
        v_r = sb.tile([128, S], mybir.dt.float32r, name="v_r")  # V, f32r
        v_stage = ExitStack()
        v_pool = v_stage.enter_context(tc.tile_pool(name="v_pool", bufs=1))
        v_sb = v_pool.tile([128, S], F16, name="v_sb")  # chunk c at cols c*H
        ccol_sb = sb.tile([128, NT], F32, name="ccol_sb")
        ident = sb.tile([128, 128], F32, name="ident")
        d_sb = sb.tile([128, NT], F32, name="d_sb")  # rowsum(E) per row-tile
        a_sb = sb.tile([128, NT], F32, name="a_sb")  # affine mult per row-tile
        b_sb = sb.tile([128, NT], F32, name="b_sb")  # affine add per row-tile
        sumv_sb = sb.tile([1, H], F32, name="sumv_sb")
        sumvb_sb = sb.tile([128, H], F32, name="sumvb_sb")
        ones1_sb = sb.tile([1, 128], F32, name="ones1_sb")

        # qt + first kt pieces on the sync queue (gate the first matmuls);
        # the rest on the gpsimd queue so issue slots don't serialize
        nc.sync.dma_start(out=qt_sb[:, :], in_=qt[:, :])
        for p in range(3):
            cols = slice(p * (S // 4), (p + 1) * (S // 4))
            nc.sync.dma_start(out=kt_sb[:, cols], in_=kt[:, cols])
        nc.gpsimd.dma_start(out=kt_sb[:, 3 * (S // 4) :], in_=kt[:, 3 * (S // 4) :])
        nc.gpsimd.dma_start(out=v_sb[:, :], in_=vc[:, :])
        nc.gpsimd.dma_start(out=ccol_sb[:, :], in_=copy_col[:, :])
        nc.gpsimd.dma_start(out=sumv_sb[0:1, :], in_=sumv_row[0:1, :])
        # round V to float32r for the full-rate context matmuls, then drop
        # the f16 staging buffer
        for p in range(4):
            cols = slice(p * (S // 4), (p + 1) * (S // 4))
            nc.vector.tensor_copy(v_r[:, cols], v_sb[:, cols])
        v_stage.close()
        make_identity(nc, ident[:, :])

        ep = ctx.enter_context(tc.tile_pool(name="ep", bufs=2))
        ctxT_sb = ep.tile([128, R], F32, name="ctxT_sb", bufs=1)

        with tc.tile_pool(name="b_ps", bufs=1, space="PSUM") as b_ps, \
             tc.tile_pool(name="c_ps", bufs=1, space="PSUM") as c_ps, \
             tc.tile_pool(name="ctx_ps", bufs=1, space="PSUM") as ctx_ps, \
             tc.tile_pool(name="b_e", bufs=3) as b_e, \
             tc.tile_pool(name="c_et", bufs=3) as c_et, \
             tc.tile_pool(name="b_small", bufs=4) as b_small:
            ps_ctx = ctx_ps.tile([128, R], F32, name="ps_ctx")

            pending_ctx = []  # (chunk_idx, et) awaiting their ctx matmuls

            def flush_ctx():
                while pending_ctx:
                    c, et = pending_ctx.pop(0)
                    vslice = v_r[:, c * 128 : (c + 1) * 128]
                    for j in range(2):
                        nc.tensor.matmul(
                            ps_ctx[:, j * 512 : (j + 1) * 512],
                            vslice,
                            et[:, j * 512 : (j + 1) * 512],
                            start=(c == 0), stop=(c == NC - 1),
                            skip_group_check=True,
                        )

            def c_chunk(c):
                # transposed-score chunk: scores^T -> exp (float32r out); the
                # E^T @ V matmuls are deferred one chunk (flush_ctx) so the
                # PE never waits on this chunk's exp.
                ps_t = c_ps.tile([128, R], F32, tag="cscore", name=f"cs{c}")
                kslice = kt_sb[:, c * 128 : (c + 1) * 128]
                for j in range(2):
                    nc.tensor.matmul(
                        ps_t[:, j * 512 : (j + 1) * 512],
                        kslice,
                        qt_sb[:, j * 512 : (j + 1) * 512],
                        start=True, stop=True,
                    )
                flush_ctx()
                et = c_et.tile([128, R], mybir.dt.float32r, tag="Et", name=f"et{c}")
                nc.scalar.activation(et[:, :], ps_t[:, :], EXP, scale=SCALE)
                pending_ctx.append((c, et))

            # Interleaved main loop: per B score block (4 matmuls + one
            # FD=2048 exp), two-ish transposed chunks. Chunks are front-
            # loaded (9/9/9/9/9/9/6/4 per row-tile) so even the last
            # row-tile's exps stay covered without a bare C-C lockstep tail.
            chunks_per_tile = [8, 8, 8, 8, 8, 8, 8, 8]
            cchunk = 0
            for t in range(NT):
                budget = chunks_per_tile[t]
                e_t = b_e.tile([128, S], F32, tag="E", name=f"e{t}")
                dparts = b_small.tile([128, 4], F32, tag="dp", name=f"dp{t}")
                qslice = qt_sb[:, t * 128 : (t + 1) * 128]
                for g in range(4):
                    ps_s = b_ps.tile([128, 2048], F32, tag="bscore", name=f"bs{t}_{g}")
                    for j in range(4):
                        col = (g * 4 + j) * 512
                        nc.tensor.matmul(
                            ps_s[:, j * 512 : (j + 1) * 512],
                            qslice,
                            kt_sb[:, col : col + 512],
                            start=True, stop=True,
                        )
                    nc.scalar.activation(
                        e_t[:, g * 2048 : (g + 1) * 2048],
                        ps_s[:, :],
                        EXP,
                        scale=SCALE,
                        accum_out=dparts[:, g : g + 1],
                    )
                    take = min(budget, 3 if g == 3 else 2)
                    for _ in range(take):
                        c_chunk(cchunk)
                        cchunk += 1
                    budget -= take
                # per-row scalars: u = 1 + S*c ; a = 1/(D*u) ; b = c/u
                dcol = d_sb[:, t : t + 1]
                nc.vector.tensor_reduce(dcol, dparts[:, :], mybir.AxisListType.X, ADD)
                u_t = b_small.tile([128, 1], F32, tag="u", name=f"u{t}")
                ru_t = b_small.tile([128, 1], F32, tag="ru", name=f"ru{t}")
                du_t = b_small.tile([128, 1], F32, tag="du", name=f"du{t}")
                ccol = ccol_sb[:, t : t + 1]
                nc.vector.tensor_scalar(u_t[:, :], ccol, float(S), 1.0, MULT, ADD)
                nc.vector.reciprocal(ru_t[:, :], u_t[:, :])
                nc.vector.tensor_mul(du_t[:, :], dcol, u_t[:, :])
                nc.vector.reciprocal(a_sb[:, t : t + 1], du_t[:, :])
                nc.vector.tensor_mul(b_sb[:, t : t + 1], ccol, ru_t[:, :])
                # alphas tile = a*E + b, in place, then write back; split in
                # four so no single DVE op blocks the pipeline for long
                for g in range(4):
                    cols = slice(g * 2048, (g + 1) * 2048)
                    nc.vector.tensor_scalar(
                        e_t[:, cols], e_t[:, cols],
                        a_sb[:, t : t + 1], b_sb[:, t : t + 1],
                        MULT, ADD,
                    )
                nc.sync.dma_start(
                    out=alphas[t * 128 : (t + 1) * 128, :], in_=e_t[:, :]
                )

            while cchunk < NC:
                c_chunk(cchunk)
                cchunk += 1
            flush_ctx()
            nc.vector.tensor_copy(ctxT_sb[:, :], ps_ctx[:, :])

        # epilogue: transpose ctx^T back to row-major, scale by a, add b*colsum(v)
        with tc.tile_pool(name="ep_ps", bufs=2, space="PSUM") as ep_ps:
            # replicate colsum(v) across partitions via a K=1 fp32 matmul
            nc.vector.memset(ones1_sb[0:1, :], 1.0)
            psvb = ep_ps.tile([128, H], F32, tag="psvb", name="psvb", bufs=1)
            nc.tensor.matmul(
                psvb[:, :], ones1_sb[0:1, :], sumv_sb[0:1, :], start=True, stop=True
            )
            nc.vector.tensor_copy(sumvb_sb[:, :], psvb[:, :])
            for t in range(NT):
                ps_tr = ep_ps.tile([128, 128], F32, tag="pstr", name=f"ptr{t}")
                nc.tensor.matmul(
                    ps_tr[:, :],
                    ctxT_sb[:, t * 128 : (t + 1) * 128],
                    ident[:, :],
                    is_transpose=True, start=True, stop=True,
                )
                co = ep.tile([128, H], F32, tag="co", name=f"co{t}")
                nc.vector.tensor_scalar(
                    co[:, :], ps_tr[:, :], a_sb[:, t : t + 1], None, MULT
                )
                # co += b * colsum(v)  (sumvb is colsum(v) replicated per partition)
                nc.vector.scalar_tensor_tensor(
                    co[:, :], sumvb_sb[:, :], b_sb[:, t : t + 1], co[:, :],
                    MULT, ADD,
                )
                nc.sync.dma_start(
                    out=ctx_out[t * 128 : (t + 1) * 128, :], in_=co[:, :]
                )

    nc.compile()
    return nc


def kernel(embedding, copy, Wq, Wk, Wv, mask, _trace=False):
    global _CACHED_NC
    if _CACHED_NC is None:
        _CACHED_NC = build()
    nc = _CACHED_NC

    emb64 = np.asarray(embedding, dtype=np.float64)
    k_full = emb64 @ np.asarray(Wk, dtype=np.float64).T  # [S, H]
    q_full = emb64 @ np.asarray(Wq, dtype=np.float64).T
    v_full = emb64 @ np.asarray(Wv, dtype=np.float64).T
    kt_np = np.ascontiguousarray(k_full.T.astype(np.float16))  # [H, S]
    # v_sb layout: [p, c*H + h] = V[c*128 + p, h]
    vc_np = np.ascontiguousarray(
        v_full.reshape(NC, 128, H).transpose(1, 0, 2).reshape(128, S).astype(np.float16)
    )
    sumv_np = v_full.sum(axis=0).astype(np.float32)[None, :]

    in_maps = []
    for m in range(M):
        rows = slice(m * R, (m + 1) * R)
        c_m = np.asarray(copy[rows], dtype=np.float32).reshape(NT, 128)
        in_maps.append(
            {
                "kt": kt_np,
                "qt": np.ascontiguousarray(q_full[rows].T.astype(np.float16)),
                "vc": vc_np,
                "copy_col": np.ascontiguousarray(c_m.T),
                "sumv_row": sumv_np,
            }
        )

    res = run_bass_kernel_spmd(nc, in_maps, core_ids=list(range(M)), trace=_trace)
    global _LAST_RES
    _LAST_RES = res
    alphas = np.concatenate([r["alphas"] for r in res.results], axis=0)
    context = np.concatenate([r["ctx"] for r in res.results], axis=0)
    if _trace:
        return (context, alphas), res
    return context, alphas


# revision 55
# speedup vs baseline: 1.1187x; 1.1058x over previous
"""Trainium2 Bass kernel for nn_CausalSelfAttention (sparse_attention).

Computes, for embedding [S=8192, H=128] and per-row copy scalars c:
    q = emb @ Wq.T ; k = emb @ Wk.T ; v = emb @ Wv.T
    scores = q @ k.T / sqrt(H)
    sm     = softmax(scores, axis=-1)
    alphas = (sm + c[:, None]) / l1norm_row(sm + c)
    context = alphas @ v
Returns (context [S, H], alphas [S, S]) as float32, matching the reference.

Key algebra: since softmax rows sum to 1 and c >= 0,
    alphas = a * E + b         with E = exp(scores/sqrt(H)),
                                    a = 1 / (D * (1 + S*c)),  D = rowsum(E),
                                    b = c / (1 + S*c)
    context = a * (E @ v) + b * colsum(v)

Sharding: rows are split across 8 NeuronCores (1024 rows each); K/V are
replicated. The tiny projections (0.3% of FLOPs) run on the host in f64;
each core runs two interleaved exp chains over its [1024, 8192] score
block: a row-major one that feeds the (memory-bound) alphas writeback and
a transposed one that feeds the E^T @ V context contraction on the PE.
The interleave keeps ScalarE (the bottleneck engine) continuously busy.
"""

import sys

if "/opt/trn_rl_repo" not in sys.path:
    sys.path.insert(0, "/opt/trn_rl_repo")

from contextlib import ExitStack

import numpy as np

import concourse.bass as bass
import concourse.tile as tile
from concourse import bacc, mybir
from concourse.bass_utils import run_bass_kernel_spmd
from concourse.masks import make_identity

S = 8192
H = 128
M = 8  # cores
R = S // M  # rows per core = 1024
NT = R // 128  # row-tiles per core = 8
NC = S // 128  # key chunks = 64
SCALE = 1.0 / float(np.sqrt(H))

F16 = mybir.dt.float16
F32 = mybir.dt.float32
EXP = mybir.ActivationFunctionType.Exp
MULT = mybir.AluOpType.mult
ADD = mybir.AluOpType.add

_CACHED_NC = None


def build():
    nc = bacc.Bacc("TRN2", target_bir_lowering=False, debug=False)

    kt = nc.dram_tensor("kt", [H, S], F16, kind="ExternalInput").ap()
    qt = nc.dram_tensor("qt", [H, R], F16, kind="ExternalInput").ap()
    vc = nc.dram_tensor("vc", [128, S], F16, kind="ExternalInput").ap()
    copy_col = nc.dram_tensor("copy_col", [128, NT], F32, kind="ExternalInput").ap()
    sumv_row = nc.dram_tensor("sumv_row", [1, H], F32, kind="ExternalInput").ap()

    alphas = nc.dram_tensor("alphas", [R, S], F32, kind="ExternalOutput").ap()
    ctx_out = nc.dram_tensor("ctx", [R, H], F32, kind="ExternalOutput").ap()

    with tile.TileContext(nc) as tc, ExitStack() as ctx:
        # ---- persistent SBUF ----
        sb = ctx.enter_context(tc.tile_pool(name="sb", bufs=1))
        kt_sb = sb.tile([H, S], F16, name="kt_sb")  # K^T: [h, s']
        qt_sb = sb.tile([H, R], F16, name="qt_sb")  # Q^T: [h, r]
        v_r = sb.tile([128, S], mybir.dt.float32r, name="v_r")  # V, f32r
        v_stage = ExitStack()
        v_pool = v_stage.enter_context(tc.tile_pool(name="v_pool", bufs=1))
        v_sb = v_pool.tile([128, S], F16, name="v_sb")  # chunk c at cols c*H
        ccol_sb = sb.tile([128, NT], F32, name="ccol_sb")
        ident = sb.tile([128, 128], F32, name="ident")
        d_sb = sb.tile([128, NT], F32, name="d_sb")  # rowsum(E) per row-tile
        a_sb = sb.tile([128, NT], F32, name="a_sb")  # affine mult per row-tile
        b_sb = sb.tile([128, NT], F32, name="b_sb")  # affine add per row-tile
        sumv_sb = sb.tile([1, H], F32, name="sumv_sb")
        sumvb_sb = sb.tile([128, H], F32, name="sumvb_sb")
        ones1_sb = sb.tile([1, 128], F32, name="ones1_sb")

        # qt + first kt pieces on the sync queue (gate the first matmuls);
        # the rest on the gpsimd queue so issue slots don't serialize
        nc.sync.dma_start(out=qt_sb[:, :], in_=qt[:, :])
        for p in range(3):
            cols = slice(p * (S // 4), (p + 1) * (S // 4))
            nc.sync.dma_start(out=kt_sb[:, cols], in_=kt[:, cols])
        nc.gpsimd.dma_start(out=kt_sb[:, 3 * (S // 4) :], in_=kt[:, 3 * (S // 4) :])
        nc.gpsimd.dma_start(out=v_sb[:, :], in_=vc[:, :])
        nc.gpsimd.dma_start(out=ccol_sb[:, :], in_=copy_col[:, :])
        nc.gpsimd.dma_start(out=sumv_sb[0:1, :], in_=sumv_row[0:1, :])
        # round V to float32r for the full-rate context matmuls, then drop
        # the f16 staging buffer
        for p in range(4):
            cols = slice(p * (S // 4), (p + 1) * (S // 4))
            nc.vector.tensor_copy(v_r[:, cols], v_sb[:, cols])
        v_stage.close()
        make_identity(nc, ident[:, :])

        ep = ctx.enter_context(tc.tile_pool(name="ep", bufs=2))
        ctx_acc = ep.tile([128, R], F32, name="ctx_acc", bufs=1)
        nc.vector.memset(ctx_acc[:, :], 0.0)

        with tc.tile_pool(name="b_ps", bufs=1, space="PSUM") as b_ps, \
             tc.tile_pool(name="c_ps", bufs=2, space="PSUM") as c_ps, \
             tc.tile_pool(name="b_e", bufs=3) as b_e, \
             tc.tile_pool(name="c_et", bufs=4) as c_et, \
             tc.tile_pool(name="b_small", bufs=4) as b_small:

            pending_ctx = []  # (chunk_idx, et) pairs awaiting ctx matmuls

            def flush_ctx():
                # 4 matmuls accumulate a chunk-pair's E^T @ V into a rotating
                # psum tile, then one DVE add folds it into ctx_acc (SBUF) —
                # no PSUM bank is held across the loop.
                if not pending_ctx:
                    return
                pair = pending_ctx[:]
                del pending_ctx[:]
                ps_p = c_ps.tile([128, R], F32, tag="cscore", name=f"pr{pair[0][0]}")
                n = len(pair)
                for idx, (c, et) in enumerate(pair):
                    vslice = v_r[:, c * 128 : (c + 1) * 128]
                    for j in range(2):
                        nc.tensor.matmul(
                            ps_p[:, j * 512 : (j + 1) * 512],
                            vslice,
                            et[:, j * 512 : (j + 1) * 512],
                            start=(idx == 0), stop=(idx == n - 1),
                            skip_group_check=True,
                        )
                nc.vector.tensor_add(ctx_acc[:, :], ctx_acc[:, :], ps_p[:, :])

            def c_chunk(c):
                # transposed-score chunk: scores^T -> exp (float32r out)
                ps_t = c_ps.tile([128, R], F32, tag="cscore", name=f"cs{c}")
                kslice = kt_sb[:, c * 128 : (c + 1) * 128]
                for j in range(2):
                    nc.tensor.matmul(
                        ps_t[:, j * 512 : (j + 1) * 512],
                        kslice,
                        qt_sb[:, j * 512 : (j + 1) * 512],
                        start=True, stop=True,
                    )
                et = c_et.tile([128, R], mybir.dt.float32r, tag="Et", name=f"et{c}")
                nc.scalar.activation(et[:, :], ps_t[:, :], EXP, scale=SCALE)
                pending_ctx.append((c, et))

            # Interleaved main loop: per B score block (4 matmuls + one
            # FD=2048 exp), two-ish transposed chunks. Chunks are front-
            # loaded (9/9/9/9/9/9/6/4 per row-tile) so even the last
            # row-tile's exps stay covered without a bare C-C lockstep tail.
            chunks_per_tile = [8, 8, 8, 8, 8, 8, 8, 8]
            cchunk = 0
            for t in range(NT):
                budget = chunks_per_tile[t]
                e_t = b_e.tile([128, S], F32, tag="E", name=f"e{t}")
                dparts = b_small.tile([128, 4], F32, tag="dp", name=f"dp{t}")
                qslice = qt_sb[:, t * 128 : (t + 1) * 128]
                for g in range(4):
                    ps_s = b_ps.tile([128, 2048], F32, tag="bscore", name=f"bs{t}_{g}")
                    for j in range(4):
                        col = (g * 4 + j) * 512
                        nc.tensor.matmul(
                            ps_s[:, j * 512 : (j + 1) * 512],
                            qslice,
                            kt_sb[:, col : col + 512],
                            start=True, stop=True,
                        )
                    flush_ctx()  # previous pair's deferred ctx matmuls
                    nc.scalar.activation(
                        e_t[:, g * 2048 : (g + 1) * 2048],
                        ps_s[:, :],
                        EXP,
                        scale=SCALE,
                        accum_out=dparts[:, g : g + 1],
                    )
                    take = min(budget, 3 if g == 3 else 2)
                    for _ in range(take):
                        c_chunk(cchunk)
                        cchunk += 1
                    budget -= take
                # per-row scalars: u = 1 + S*c ; a = 1/(D*u) ; b = c/u
                dcol = d_sb[:, t : t + 1]
                nc.vector.tensor_reduce(dcol, dparts[:, :], mybir.AxisListType.X, ADD)
                u_t = b_small.tile([128, 1], F32, tag="u", name=f"u{t}")
                ru_t = b_small.tile([128, 1], F32, tag="ru", name=f"ru{t}")
                du_t = b_small.tile([128, 1], F32, tag="du", name=f"du{t}")
                ccol = ccol_sb[:, t : t + 1]
                nc.vector.tensor_scalar(u_t[:, :], ccol, float(S), 1.0, MULT, ADD)
                nc.vector.reciprocal(ru_t[:, :], u_t[:, :])
                nc.vector.tensor_mul(du_t[:, :], dcol, u_t[:, :])
                nc.vector.reciprocal(a_sb[:, t : t + 1], du_t[:, :])
                nc.vector.tensor_mul(b_sb[:, t : t + 1], ccol, ru_t[:, :])
                # alphas tile = a*E + b, in place, then write back; split in
                # four so no single DVE op blocks the pipeline for long
                for g in range(4):
                    cols = slice(g * 2048, (g + 1) * 2048)
                    nc.vector.tensor_scalar(
                        e_t[:, cols], e_t[:, cols],
                        a_sb[:, t : t + 1], b_sb[:, t : t + 1],
                        MULT, ADD,
                    )
                nc.sync.dma_start(
                    out=alphas[t * 128 : (t + 1) * 128, :], in_=e_t[:, :]
                )

            while cchunk < NC:
                c_chunk(cchunk)
                cchunk += 1
            flush_ctx()

        # epilogue: transpose ctx^T back to row-major, scale by a, add b*colsum(v)
        with tc.tile_pool(name="ep_ps", bufs=2, space="PSUM") as ep_ps:
            # replicate colsum(v) across partitions via a K=1 fp32 matmul
            nc.vector.memset(ones1_sb[0:1, :], 1.0)
            psvb = ep_ps.tile([128, H], F32, tag="psvb", name="psvb", bufs=1)
            nc.tensor.matmul(
                psvb[:, :], ones1_sb[0:1, :], sumv_sb[0:1, :], start=True, stop=True
            )
            nc.vector.tensor_copy(sumvb_sb[:, :], psvb[:, :])
            for t in range(NT):
                ps_tr = ep_ps.tile([128, 128], F32, tag="pstr", name=f"ptr{t}")
                nc.tensor.matmul(
                    ps_tr[:, :],
                    ctx_acc[:, t * 128 : (t + 1) * 128],
                    ident[:, :],
                    is_transpose=True, start=True, stop=True,
                )
                co = ep.tile([128, H], F32, tag="co", name=f"co{t}")
                nc.vector.tensor_scalar(
                    co[:, :], ps_tr[:, :], a_sb[:, t : t + 1], None, MULT
                )
                # co += b * colsum(v)  (sumvb is colsum(v) replicated per partition)
                nc.vector.scalar_tensor_tensor(
                    co[:, :], sumvb_sb[:, :], b_sb[:, t : t + 1], co[:, :],
                    MULT, ADD,
                )
                nc.sync.dma_start(
                    out=ctx_out[t * 128 : (t + 1) * 128, :], in_=co[:, :]
                )

    nc.compile()
    return nc


def kernel(embedding, copy, Wq, Wk, Wv, mask, _trace=False):
    global _CACHED_NC
    if _CACHED_NC is None:
        _CACHED_NC = build()
    nc = _CACHED_NC

    emb64 = np.asarray(embedding, dtype=np.float64)
    k_full = emb64 @ np.asarray(Wk, dtype=np.float64).T  # [S, H]
    q_full = emb64 @ np.asarray(Wq, dtype=np.float64).T
    v_full = emb64 @ np.asarray(Wv, dtype=np.float64).T
    kt_np = np.ascontiguousarray(k_full.T.astype(np.float16))  # [H, S]
    # v_sb layout: [p, c*H + h] = V[c*128 + p, h]
    vc_np = np.ascontiguousarray(
        v_full.reshape(NC, 128, H).transpose(1, 0, 2).reshape(128, S).astype(np.float16)
    )
    sumv_np = v_full.sum(axis=0).astype(np.float32)[None, :]

    in_maps = []
    for m in range(M):
        rows = slice(m * R, (m + 1) * R)
        c_m = np.asarray(copy[rows], dtype=np.float32).reshape(NT, 128)
        in_maps.append(
            {
                "kt": kt_np,
                "qt": np.ascontiguousarray(q_full[rows].T.astype(np.float16)),
                "vc": vc_np,
                "copy_col": np.ascontiguousarray(c_m.T),
                "sumv_row": sumv_np,
            }
        )

    res = run_bass_kernel_spmd(nc, in_maps, core_ids=list(range(M)), trace=_trace)
    global _LAST_RES
    _LAST_RES = res
    alphas = np.concatenate([r["alphas"] for r in res.results], axis=0)
    context = np.concatenate([r["ctx"] for r in res.results], axis=0)
    if _trace:
        return (context, alphas), res
    return context, alphas


# revision 58
# speedup vs baseline: 1.1234x; 1.0042x over previous
"""Trainium2 Bass kernel for nn_CausalSelfAttention (sparse_attention).

Computes, for embedding [S=8192, H=128] and per-row copy scalars c:
    q = emb @ Wq.T ; k = emb @ Wk.T ; v = emb @ Wv.T
    scores = q @ k.T / sqrt(H)
    sm     = softmax(scores, axis=-1)
    alphas = (sm + c[:, None]) / l1norm_row(sm + c)
    context = alphas @ v
Returns (context [S, H], alphas [S, S]) as float32, matching the reference.

Key algebra: since softmax rows sum to 1 and c >= 0,
    alphas = a * E + b         with E = exp(scores/sqrt(H)),
                                    a = 1 / (D * (1 + S*c)),  D = rowsum(E),
                                    b = c / (1 + S*c)
    context = a * (E @ v) + b * colsum(v)

Sharding: rows are split across 8 NeuronCores (1024 rows each); K/V are
replicated. The tiny projections (0.3% of FLOPs) run on the host in f64;
each core runs two interleaved exp chains over its [1024, 8192] score
block: a row-major one that feeds the (memory-bound) alphas writeback and
a transposed one that feeds the E^T @ V context contraction on the PE.
The interleave keeps ScalarE (the bottleneck engine) continuously busy.
"""

import sys

if "/opt/trn_rl_repo" not in sys.path:
    sys.path.insert(0, "/opt/trn_rl_repo")

from contextlib import ExitStack

import numpy as np

import concourse.bass as bass
import concourse.tile as tile
from concourse import bacc, mybir
from concourse.bass_utils import run_bass_kernel_spmd
from concourse.masks import make_identity

S = 8192
H = 128
M = 8  # cores
R = S // M  # rows per core = 1024
NT = R // 128  # row-tiles per core = 8
NC = S // 128  # key chunks = 64
SCALE = 1.0 / float(np.sqrt(H))

F16 = mybir.dt.float16
F32 = mybir.dt.float32
EXP = mybir.ActivationFunctionType.Exp
MULT = mybir.AluOpType.mult
ADD = mybir.AluOpType.add

_CACHED_NC = None


def build():
    nc = bacc.Bacc("TRN2", target_bir_lowering=False, debug=False)

    kt = nc.dram_tensor("kt", [H, S], F16, kind="ExternalInput").ap()
    qt = nc.dram_tensor("qt", [H, R], F16, kind="ExternalInput").ap()
    vc = nc.dram_tensor("vc", [128, S], F16, kind="ExternalInput").ap()
    copy_col = nc.dram_tensor("copy_col", [128, NT], F32, kind="ExternalInput").ap()
    sumv_row = nc.dram_tensor("sumv_row", [1, H], F32, kind="ExternalInput").ap()

    alphas = nc.dram_tensor("alphas", [R, S], F32, kind="ExternalOutput").ap()
    ctx_out = nc.dram_tensor("ctx", [R, H], F32, kind="ExternalOutput").ap()

    with tile.TileContext(nc) as tc, ExitStack() as ctx:
        # ---- persistent SBUF ----
        sb = ctx.enter_context(tc.tile_pool(name="sb", bufs=1))
        kt_sb = sb.tile([H, S], F16, name="kt_sb")  # K^T: [h, s']
        qt_sb = sb.tile([H, R], F16, name="qt_sb")  # Q^T: [h, r]
        v_r = sb.tile([128, S], mybir.dt.float32r, name="v_r")  # V, f32r
        v_stage = ExitStack()
        v_pool = v_stage.enter_context(tc.tile_pool(name="v_pool", bufs=1))
        v_sb = v_pool.tile([128, S], F16, name="v_sb")  # chunk c at cols c*H
        ccol_sb = sb.tile([128, NT], F32, name="ccol_sb")
        ident = sb.tile([128, 128], F32, name="ident")
        d_sb = sb.tile([128, NT], F32, name="d_sb")  # rowsum(E) per row-tile
        a_sb = sb.tile([128, NT], F32, name="a_sb")  # affine mult per row-tile
        b_sb = sb.tile([128, NT], F32, name="b_sb")  # affine add per row-tile
        sumv_sb = sb.tile([1, H], F32, name="sumv_sb")
        sumvb_sb = sb.tile([128, H], F32, name="sumvb_sb")
        ones1_sb = sb.tile([1, 128], F32, name="ones1_sb")

        # qt + first kt pieces on the sync queue (gate the first matmuls);
        # the rest on the gpsimd queue so issue slots don't serialize
        nc.sync.dma_start(out=qt_sb[:, :], in_=qt[:, :])
        for p in range(3):
            cols = slice(p * (S // 4), (p + 1) * (S // 4))
            nc.sync.dma_start(out=kt_sb[:, cols], in_=kt[:, cols])
        nc.gpsimd.dma_start(out=kt_sb[:, 3 * (S // 4) :], in_=kt[:, 3 * (S // 4) :])
        nc.gpsimd.dma_start(out=v_sb[:, :], in_=vc[:, :])
        nc.gpsimd.dma_start(out=ccol_sb[:, :], in_=copy_col[:, :])
        nc.gpsimd.dma_start(out=sumv_sb[0:1, :], in_=sumv_row[0:1, :])
        # round V to float32r for the full-rate context matmuls, then drop
        # the f16 staging buffer
        for p in range(4):
            cols = slice(p * (S // 4), (p + 1) * (S // 4))
            nc.vector.tensor_copy(v_r[:, cols], v_sb[:, cols])
        v_stage.close()
        make_identity(nc, ident[:, :])

        ep = ctx.enter_context(tc.tile_pool(name="ep", bufs=2))
        ctx_acc = ep.tile([128, R], F32, name="ctx_acc", bufs=1)
        nc.vector.memset(ctx_acc[:, :], 0.0)

        with tc.tile_pool(name="b_ps", bufs=1, space="PSUM") as b_ps, \
             tc.tile_pool(name="c_ps", bufs=2, space="PSUM") as c_ps, \
             tc.tile_pool(name="b_e", bufs=3) as b_e, \
             tc.tile_pool(name="c_et", bufs=4) as c_et, \
             tc.tile_pool(name="b_small", bufs=4) as b_small:

            pending_ctx = []  # (chunk_idx, et) pairs awaiting ctx matmuls

            def flush_ctx():
                # 4 matmuls accumulate a chunk-pair's E^T @ V into a rotating
                # psum tile, then one DVE add folds it into ctx_acc (SBUF) —
                # no PSUM bank is held across the loop.
                if not pending_ctx:
                    return
                pair = pending_ctx[:]
                del pending_ctx[:]
                ps_p = c_ps.tile([128, R], F32, tag="cscore", name=f"pr{pair[0][0]}")
                n = len(pair)
                for idx, (c, et) in enumerate(pair):
                    vslice = v_r[:, c * 128 : (c + 1) * 128]
                    for j in range(2):
                        nc.tensor.matmul(
                            ps_p[:, j * 512 : (j + 1) * 512],
                            vslice,
                            et[:, j * 512 : (j + 1) * 512],
                            start=(idx == 0), stop=(idx == n - 1),
                            skip_group_check=True,
                        )
                nc.vector.tensor_add(ctx_acc[:, :], ctx_acc[:, :], ps_p[:, :])

            def c_chunk(c):
                # transposed-score chunk: scores^T -> exp (float32r out)
                ps_t = c_ps.tile([128, R], F32, tag="cscore", name=f"cs{c}")
                kslice = kt_sb[:, c * 128 : (c + 1) * 128]
                for j in range(2):
                    nc.tensor.matmul(
                        ps_t[:, j * 512 : (j + 1) * 512],
                        kslice,
                        qt_sb[:, j * 512 : (j + 1) * 512],
                        start=True, stop=True,
                    )
                et = c_et.tile([128, R], mybir.dt.float32r, tag="Et", name=f"et{c}")
                nc.scalar.activation(et[:, :], ps_t[:, :], EXP, scale=SCALE)
                pending_ctx.append((c, et))

            # Interleaved main loop: per B score block (4 matmuls + one
            # FD=2048 exp), two-ish transposed chunks. Chunks are front-
            # loaded (9/9/9/9/9/9/6/4 per row-tile) so even the last
            # row-tile's exps stay covered without a bare C-C lockstep tail.
            chunks_per_tile = [2, 4, 6, 8, 10, 10, 12, 12]
            cchunk = 0
            for t in range(NT):
                budget = chunks_per_tile[t]
                e_t = b_e.tile([128, S], F32, tag="E", name=f"e{t}")
                dparts = b_small.tile([128, 4], F32, tag="dp", name=f"dp{t}")
                qslice = qt_sb[:, t * 128 : (t + 1) * 128]
                for g in range(4):
                    ps_s = b_ps.tile([128, 2048], F32, tag="bscore", name=f"bs{t}_{g}")
                    for j in range(4):
                        col = (g * 4 + j) * 512
                        nc.tensor.matmul(
                            ps_s[:, j * 512 : (j + 1) * 512],
                            qslice,
                            kt_sb[:, col : col + 512],
                            start=True, stop=True,
                        )
                    flush_ctx()  # previous pair's deferred ctx matmuls
                    nc.scalar.activation(
                        e_t[:, g * 2048 : (g + 1) * 2048],
                        ps_s[:, :],
                        EXP,
                        scale=SCALE,
                        accum_out=dparts[:, g : g + 1],
                    )
                    take = budget if g == 3 else min(budget, 2)
                    for _ in range(take):
                        c_chunk(cchunk)
                        cchunk += 1
                    budget -= take
                # per-row scalars: u = 1 + S*c ; a = 1/(D*u) ; b = c/u
                dcol = d_sb[:, t : t + 1]
                nc.vector.tensor_reduce(dcol, dparts[:, :], mybir.AxisListType.X, ADD)
                u_t = b_small.tile([128, 1], F32, tag="u", name=f"u{t}")
                ru_t = b_small.tile([128, 1], F32, tag="ru", name=f"ru{t}")
                du_t = b_small.tile([128, 1], F32, tag="du", name=f"du{t}")
                ccol = ccol_sb[:, t : t + 1]
                nc.vector.tensor_scalar(u_t[:, :], ccol, float(S), 1.0, MULT, ADD)
                nc.vector.reciprocal(ru_t[:, :], u_t[:, :])
                nc.vector.tensor_mul(du_t[:, :], dcol, u_t[:, :])
                nc.vector.reciprocal(a_sb[:, t : t + 1], du_t[:, :])
                nc.vector.tensor_mul(b_sb[:, t : t + 1], ccol, ru_t[:, :])
                # alphas tile = a*E + b, in place, then write back; split in
                # four so no single DVE op blocks the pipeline for long
                for g in range(4):
                    cols = slice(g * 2048, (g + 1) * 2048)
                    nc.vector.tensor_scalar(
                        e_t[:, cols], e_t[:, cols],
                        a_sb[:, t : t + 1], b_sb[:, t : t + 1],
                        MULT, ADD,
                    )
                nc.sync.dma_start(
                    out=alphas[t * 128 : (t + 1) * 128, :], in_=e_t[:, :]
                )

            while cchunk < NC:
                c_chunk(cchunk)
                cchunk += 1
            flush_ctx()

        # epilogue: transpose ctx^T back to row-major, scale by a, add b*colsum(v)
        with tc.tile_pool(name="ep_ps", bufs=4, space="PSUM") as ep_ps:
            # replicate colsum(v) across partitions via a K=1 fp32 matmul
            nc.vector.memset(ones1_sb[0:1, :], 1.0)
            psvb = ep_ps.tile([128, H], F32, tag="psvb", name="psvb", bufs=1)
            nc.tensor.matmul(
                psvb[:, :], ones1_sb[0:1, :], sumv_sb[0:1, :], start=True, stop=True
            )
            nc.vector.tensor_copy(sumvb_sb[:, :], psvb[:, :])
            for t in range(NT):
                ps_tr = ep_ps.tile([128, 128], F32, tag="pstr", name=f"ptr{t}")
                nc.tensor.matmul(
                    ps_tr[:, :],
                    ctx_acc[:, t * 128 : (t + 1) * 128],
                    ident[:, :],
                    is_transpose=True, start=True, stop=True,
                )
                co = ep.tile([128, H], F32, tag="co", name=f"co{t}", bufs=4)
                nc.vector.tensor_scalar(
                    co[:, :], ps_tr[:, :], a_sb[:, t : t + 1], None, MULT
                )
                # co += b * colsum(v)  (sumvb is colsum(v) replicated per partition)
                nc.vector.scalar_tensor_tensor(
                    co[:, :], sumvb_sb[:, :], b_sb[:, t : t + 1], co[:, :],
                    MULT, ADD,
                )
                # gpsimd queue: don't queue behind the big alphas DMAs
                nc.gpsimd.dma_start(
                    out=ctx_out[t * 128 : (t + 1) * 128, :], in_=co[:, :]
                )

    nc.compile()
    return nc


def kernel(embedding, copy, Wq, Wk, Wv, mask, _trace=False):
    global _CACHED_NC
    if _CACHED_NC is None:
        _CACHED_NC = build()
    nc = _CACHED_NC

    emb64 = np.asarray(embedding, dtype=np.float64)
    k_full = emb64 @ np.asarray(Wk, dtype=np.float64).T  # [S, H]
    q_full = emb64 @ np.asarray(Wq, dtype=np.float64).T
    v_full = emb64 @ np.asarray(Wv, dtype=np.float64).T
    kt_np = np.ascontiguousarray(k_full.T.astype(np.float16))  # [H, S]
    # v_sb layout: [p, c*H + h] = V[c*128 + p, h]
    vc_np = np.ascontiguousarray(
        v_full.reshape(NC, 128, H).transpose(1, 0, 2).reshape(128, S).astype(np.float16)
    )
    sumv_np = v_full.sum(axis=0).astype(np.float32)[None, :]

    in_maps = []
    for m in range(M):
        rows = slice(m * R, (m + 1) * R)
        c_m = np.asarray(copy[rows], dtype=np.float32).reshape(NT, 128)
        in_maps.append(
            {
                "kt": kt_np,
                "qt": np.ascontiguousarray(q_full[rows].T.astype(np.float16)),
                "vc": vc_np,
                "copy_col": np.ascontiguousarray(c_m.T),
                "sumv_row": sumv_np,
            }
        )

    res = run_bass_kernel_spmd(nc, in_maps, core_ids=list(range(M)), trace=_trace)
    global _LAST_RES
    _LAST_RES = res
    alphas = np.concatenate([r["alphas"] for r in res.results], axis=0)
    context = np.concatenate([r["ctx"] for r in res.results], axis=0)
    if _trace:
        return (context, alphas), res
    return context, alphas


# revision 59
# speedup vs baseline: 1.1242x; 1.0007x over previous
"""Trainium2 Bass kernel for nn_CausalSelfAttention (sparse_attention).

Computes, for embedding [S=8192, H=128] and per-row copy scalars c:
    q = emb @ Wq.T ; k = emb @ Wk.T ; v = emb @ Wv.T
    scores = q @ k.T / sqrt(H)
    sm     = softmax(scores, axis=-1)
    alphas = (sm + c[:, None]) / l1norm_row(sm + c)
    context = alphas @ v
Returns (context [S, H], alphas [S, S]) as float32, matching the reference.

Key algebra: since softmax rows sum to 1 and c >= 0,
    alphas = a * E + b         with E = exp(scores/sqrt(H)),
                                    a = 1 / (D * (1 + S*c)),  D = rowsum(E),
                                    b = c / (1 + S*c)
    context = a * (E @ v) + b * colsum(v)

Sharding: rows are split across 8 NeuronCores (1024 rows each); K/V are
replicated. The tiny projections (0.3% of FLOPs) run on the host in f64;
each core runs two interleaved exp chains over its [1024, 8192] score
block: a row-major one that feeds the (memory-bound) alphas writeback and
a transposed one that feeds the E^T @ V context contraction on the PE.
The interleave keeps ScalarE (the bottleneck engine) continuously busy.
"""

import sys

if "/opt/trn_rl_repo" not in sys.path:
    sys.path.insert(0, "/opt/trn_rl_repo")

from contextlib import ExitStack

import numpy as np

import concourse.bass as bass
import concourse.tile as tile
from concourse import bacc, mybir
from concourse.bass_utils import run_bass_kernel_spmd
from concourse.masks import make_identity

S = 8192
H = 128
M = 8  # cores
R = S // M  # rows per core = 1024
NT = R // 128  # row-tiles per core = 8
NC = S // 128  # key chunks = 64
SCALE = 1.0 / float(np.sqrt(H))

F16 = mybir.dt.float16
F32 = mybir.dt.float32
EXP = mybir.ActivationFunctionType.Exp
MULT = mybir.AluOpType.mult
ADD = mybir.AluOpType.add

_CACHED_NC = None


def build():
    nc = bacc.Bacc("TRN2", target_bir_lowering=False, debug=False)

    kt = nc.dram_tensor("kt", [H, S], F16, kind="ExternalInput").ap()
    qt = nc.dram_tensor("qt", [H, R], F16, kind="ExternalInput").ap()
    vc = nc.dram_tensor("vc", [128, S], F16, kind="ExternalInput").ap()
    copy_col = nc.dram_tensor("copy_col", [128, NT], F32, kind="ExternalInput").ap()
    sumv_row = nc.dram_tensor("sumv_row", [1, H], F32, kind="ExternalInput").ap()

    alphas = nc.dram_tensor("alphas", [R, S], F32, kind="ExternalOutput").ap()
    ctx_out = nc.dram_tensor("ctx", [R, H], F32, kind="ExternalOutput").ap()

    with tile.TileContext(nc) as tc, ExitStack() as ctx:
        # ---- persistent SBUF ----
        sb = ctx.enter_context(tc.tile_pool(name="sb", bufs=1))
        kt_sb = sb.tile([H, S], F16, name="kt_sb")  # K^T: [h, s']
        qt_sb = sb.tile([H, R], F16, name="qt_sb")  # Q^T: [h, r]
        v_r = sb.tile([128, S], mybir.dt.float32r, name="v_r")  # V, f32r
        v_stage = ExitStack()
        v_pool = v_stage.enter_context(tc.tile_pool(name="v_pool", bufs=1))
        v_sb = v_pool.tile([128, S], F16, name="v_sb")  # chunk c at cols c*H
        ccol_sb = sb.tile([128, NT], F32, name="ccol_sb")
        ident = sb.tile([128, 128], F32, name="ident")
        d_sb = sb.tile([128, NT], F32, name="d_sb")  # rowsum(E) per row-tile
        a_sb = sb.tile([128, NT], F32, name="a_sb")  # affine mult per row-tile
        b_sb = sb.tile([128, NT], F32, name="b_sb")  # affine add per row-tile
        sumv_sb = sb.tile([1, H], F32, name="sumv_sb")
        sumvb_sb = sb.tile([128, H], F32, name="sumvb_sb")
        ones1_sb = sb.tile([1, 128], F32, name="ones1_sb")

        # qt + first kt pieces on the sync queue (gate the first matmuls);
        # the rest on the gpsimd queue so issue slots don't serialize
        nc.sync.dma_start(out=qt_sb[:, :], in_=qt[:, :])
        for p in range(3):
            cols = slice(p * (S // 4), (p + 1) * (S // 4))
            nc.sync.dma_start(out=kt_sb[:, cols], in_=kt[:, cols])
        nc.gpsimd.dma_start(out=kt_sb[:, 3 * (S // 4) :], in_=kt[:, 3 * (S // 4) :])
        nc.gpsimd.dma_start(out=v_sb[:, :], in_=vc[:, :])
        nc.gpsimd.dma_start(out=ccol_sb[:, :], in_=copy_col[:, :])
        nc.gpsimd.dma_start(out=sumv_sb[0:1, :], in_=sumv_row[0:1, :])
        # round V to float32r for the full-rate context matmuls, then drop
        # the f16 staging buffer
        for p in range(4):
            cols = slice(p * (S // 4), (p + 1) * (S // 4))
            nc.vector.tensor_copy(v_r[:, cols], v_sb[:, cols])
        v_stage.close()
        make_identity(nc, ident[:, :])

        ep = ctx.enter_context(tc.tile_pool(name="ep", bufs=2))
        ctx_acc = ep.tile([128, R], F32, name="ctx_acc", bufs=1)
        nc.vector.memset(ctx_acc[:, :], 0.0)

        with tc.tile_pool(name="b_ps", bufs=1, space="PSUM") as b_ps, \
             tc.tile_pool(name="c_ps", bufs=2, space="PSUM") as c_ps, \
             tc.tile_pool(name="b_e", bufs=3) as b_e, \
             tc.tile_pool(name="c_et", bufs=4) as c_et, \
             tc.tile_pool(name="b_small", bufs=4) as b_small:

            pending_ctx = []  # (chunk_idx, et) pairs awaiting ctx matmuls

            def flush_ctx():
                # 4 matmuls accumulate a chunk-pair's E^T @ V into a rotating
                # psum tile, then one DVE add folds it into ctx_acc (SBUF) —
                # no PSUM bank is held across the loop.
                if not pending_ctx:
                    return
                pair = pending_ctx[:]
                del pending_ctx[:]
                ps_p = c_ps.tile([128, R], F32, tag="cscore", name=f"pr{pair[0][0]}")
                n = len(pair)
                for idx, (c, et) in enumerate(pair):
                    vslice = v_r[:, c * 128 : (c + 1) * 128]
                    for j in range(2):
                        nc.tensor.matmul(
                            ps_p[:, j * 512 : (j + 1) * 512],
                            vslice,
                            et[:, j * 512 : (j + 1) * 512],
                            start=(idx == 0), stop=(idx == n - 1),
                            skip_group_check=True,
                        )
                nc.vector.tensor_add(ctx_acc[:, :], ctx_acc[:, :], ps_p[:, :])

            def c_chunk(c):
                # transposed-score chunk: scores^T -> exp (float32r out)
                ps_t = c_ps.tile([128, R], F32, tag="cscore", name=f"cs{c}")
                kslice = kt_sb[:, c * 128 : (c + 1) * 128]
                for j in range(2):
                    nc.tensor.matmul(
                        ps_t[:, j * 512 : (j + 1) * 512],
                        kslice,
                        qt_sb[:, j * 512 : (j + 1) * 512],
                        start=True, stop=True,
                    )
                et = c_et.tile([128, R], mybir.dt.float32r, tag="Et", name=f"et{c}")
                nc.scalar.activation(et[:, :], ps_t[:, :], EXP, scale=SCALE)
                pending_ctx.append((c, et))
                if len(pending_ctx) >= 2:
                    flush_ctx()

            # Interleaved main loop: per B score block (4 matmuls + one
            # FD=2048 exp), two-ish transposed chunks. Chunks are front-
            # loaded (9/9/9/9/9/9/6/4 per row-tile) so even the last
            # row-tile's exps stay covered without a bare C-C lockstep tail.
            chunks_per_tile = [2, 4, 6, 8, 10, 10, 12, 12]
            cchunk = 0
            for t in range(NT):
                budget = chunks_per_tile[t]
                e_t = b_e.tile([128, S], F32, tag="E", name=f"e{t}")
                dparts = b_small.tile([128, 4], F32, tag="dp", name=f"dp{t}")
                qslice = qt_sb[:, t * 128 : (t + 1) * 128]
                for g in range(4):
                    ps_s = b_ps.tile([128, 2048], F32, tag="bscore", name=f"bs{t}_{g}")
                    for j in range(4):
                        col = (g * 4 + j) * 512
                        nc.tensor.matmul(
                            ps_s[:, j * 512 : (j + 1) * 512],
                            qslice,
                            kt_sb[:, col : col + 512],
                            start=True, stop=True,
                        )
                    flush_ctx()  # previous pair's deferred ctx matmuls
                    nc.scalar.activation(
                        e_t[:, g * 2048 : (g + 1) * 2048],
                        ps_s[:, :],
                        EXP,
                        scale=SCALE,
                        accum_out=dparts[:, g : g + 1],
                    )
                    take = budget if g == 3 else min(budget, 2)
                    for _ in range(take):
                        c_chunk(cchunk)
                        cchunk += 1
                    budget -= take
                # per-row scalars: u = 1 + S*c ; a = 1/(D*u) ; b = c/u
                dcol = d_sb[:, t : t + 1]
                nc.vector.tensor_reduce(dcol, dparts[:, :], mybir.AxisListType.X, ADD)
                u_t = b_small.tile([128, 1], F32, tag="u", name=f"u{t}")
                ru_t = b_small.tile([128, 1], F32, tag="ru", name=f"ru{t}")
                du_t = b_small.tile([128, 1], F32, tag="du", name=f"du{t}")
                ccol = ccol_sb[:, t : t + 1]
                nc.vector.tensor_scalar(u_t[:, :], ccol, float(S), 1.0, MULT, ADD)
                nc.vector.reciprocal(ru_t[:, :], u_t[:, :])
                nc.vector.tensor_mul(du_t[:, :], dcol, u_t[:, :])
                nc.vector.reciprocal(a_sb[:, t : t + 1], du_t[:, :])
                nc.vector.tensor_mul(b_sb[:, t : t + 1], ccol, ru_t[:, :])
                # alphas tile = a*E + b, in place, then write back; split in
                # four so no single DVE op blocks the pipeline for long
                for g in range(4):
                    cols = slice(g * 2048, (g + 1) * 2048)
                    nc.vector.tensor_scalar(
                        e_t[:, cols], e_t[:, cols],
                        a_sb[:, t : t + 1], b_sb[:, t : t + 1],
                        MULT, ADD,
                    )
                nc.sync.dma_start(
                    out=alphas[t * 128 : (t + 1) * 128, :], in_=e_t[:, :]
                )

            while cchunk < NC:
                c_chunk(cchunk)
                cchunk += 1
            flush_ctx()

        # epilogue: transpose ctx^T back to row-major, scale by a, add b*colsum(v)
        with tc.tile_pool(name="ep_ps", bufs=4, space="PSUM") as ep_ps:
            # replicate colsum(v) across partitions via a K=1 fp32 matmul
            nc.vector.memset(ones1_sb[0:1, :], 1.0)
            psvb = ep_ps.tile([128, H], F32, tag="psvb", name="psvb", bufs=1)
            nc.tensor.matmul(
                psvb[:, :], ones1_sb[0:1, :], sumv_sb[0:1, :], start=True, stop=True
            )
            nc.vector.tensor_copy(sumvb_sb[:, :], psvb[:, :])
            for t in range(NT):
                ps_tr = ep_ps.tile([128, 128], F32, tag="pstr", name=f"ptr{t}")
                nc.tensor.matmul(
                    ps_tr[:, :],
                    ctx_acc[:, t * 128 : (t + 1) * 128],
                    ident[:, :],
                    is_transpose=True, start=True, stop=True,
                )
                co = ep.tile([128, H], F32, tag="co", name=f"co{t}", bufs=4)
                nc.vector.tensor_scalar(
                    co[:, :], ps_tr[:, :], a_sb[:, t : t + 1], None, MULT
                )
                # co += b * colsum(v)  (sumvb is colsum(v) replicated per partition)
                nc.vector.scalar_tensor_tensor(
                    co[:, :], sumvb_sb[:, :], b_sb[:, t : t + 1], co[:, :],
                    MULT, ADD,
                )
                # gpsimd queue: don't queue behind the big alphas DMAs
                nc.gpsimd.dma_start(
                    out=ctx_out[t * 128 : (t + 1) * 128, :], in_=co[:, :]
                )

    nc.compile()
    return nc


def kernel(embedding, copy, Wq, Wk, Wv, mask, _trace=False):
    global _CACHED_NC
    if _CACHED_NC is None:
        _CACHED_NC = build()
    nc = _CACHED_NC

    emb64 = np.asarray(embedding, dtype=np.float64)
    k_full = emb64 @ np.asarray(Wk, dtype=np.float64).T  # [S, H]
    q_full = emb64 @ np.asarray(Wq, dtype=np.float64).T
    v_full = emb64 @ np.asarray(Wv, dtype=np.float64).T
    kt_np = np.ascontiguousarray(k_full.T.astype(np.float16))  # [H, S]
    # v_sb layout: [p, c*H + h] = V[c*128 + p, h]
    vc_np = np.ascontiguousarray(
        v_full.reshape(NC, 128, H).transpose(1, 0, 2).reshape(128, S).astype(np.float16)
    )
    sumv_np = v_full.sum(axis=0).astype(np.float32)[None, :]

    in_maps = []
    for m in range(M):
        rows = slice(m * R, (m + 1) * R)
        c_m = np.asarray(copy[rows], dtype=np.float32).reshape(NT, 128)
        in_maps.append(
            {
                "kt": kt_np,
                "qt": np.ascontiguousarray(q_full[rows].T.astype(np.float16)),
                "vc": vc_np,
                "copy_col": np.ascontiguousarray(c_m.T),
                "sumv_row": sumv_np,
            }
        )

    res = run_bass_kernel_spmd(nc, in_maps, core_ids=list(range(M)), trace=_trace)
    global _LAST_RES
    _LAST_RES = res
    alphas = np.concatenate([r["alphas"] for r in res.results], axis=0)
    context = np.concatenate([r["ctx"] for r in res.results], axis=0)
    if _trace:
        return (context, alphas), res
    return context, alphas


# revision 63
# speedup vs baseline: 1.1348x; 1.0094x over previous
"""Trainium2 Bass kernel for nn_CausalSelfAttention (sparse_attention).

Computes, for embedding [S=8192, H=128] and per-row copy scalars c:
    q = emb @ Wq.T ; k = emb @ Wk.T ; v = emb @ Wv.T
    scores = q @ k.T / sqrt(H)
    sm     = softmax(scores, axis=-1)
    alphas = (sm + c[:, None]) / l1norm_row(sm + c)
    context = alphas @ v
Returns (context [S, H], alphas [S, S]) as float32, matching the reference.

Key algebra: since softmax rows sum to 1 and c >= 0,
    alphas = a * E + b         with E = exp(scores/sqrt(H)),
                                    a = 1 / (D * (1 + S*c)),  D = rowsum(E),
                                    b = c / (1 + S*c)
    context = a * (E @ v) + b * colsum(v)

Sharding: rows are split across 8 NeuronCores (1024 rows each); K/V are
replicated. The tiny projections (0.3% of FLOPs) run on the host in f64;
each core runs two interleaved exp chains over its [1024, 8192] score
block: a row-major one that feeds the (memory-bound) alphas writeback and
a transposed one that feeds the E^T @ V context contraction on the PE.
The interleave keeps ScalarE (the bottleneck engine) continuously busy.
"""

import sys

if "/opt/trn_rl_repo" not in sys.path:
    sys.path.insert(0, "/opt/trn_rl_repo")

from contextlib import ExitStack

import numpy as np

import concourse.bass as bass
import concourse.tile as tile
from concourse import bacc, mybir
from concourse.bass_utils import run_bass_kernel_spmd
from concourse.masks import make_identity

S = 8192
H = 128
M = 8  # cores
R = S // M  # rows per core = 1024
NT = R // 128  # row-tiles per core = 8
NC = S // 128  # key chunks = 64
SCALE = 1.0 / float(np.sqrt(H))

F16 = mybir.dt.float16
F32 = mybir.dt.float32
EXP = mybir.ActivationFunctionType.Exp
MULT = mybir.AluOpType.mult
ADD = mybir.AluOpType.add

_CACHED_NC = None


def build():
    nc = bacc.Bacc("TRN2", target_bir_lowering=False, debug=False)

    kt = nc.dram_tensor("kt", [H, S], F16, kind="ExternalInput").ap()
    qt = nc.dram_tensor("qt", [H, R], F16, kind="ExternalInput").ap()
    vc = nc.dram_tensor("vc", [128, S], F16, kind="ExternalInput").ap()
    copy_col = nc.dram_tensor("copy_col", [128, NT], F32, kind="ExternalInput").ap()
    sumv_row = nc.dram_tensor("sumv_row", [1, H], F32, kind="ExternalInput").ap()

    alphas = nc.dram_tensor("alphas", [R, S], F32, kind="ExternalOutput").ap()
    ctx_out = nc.dram_tensor("ctx", [R, H], F32, kind="ExternalOutput").ap()

    with tile.TileContext(nc) as tc, ExitStack() as ctx:
        # ---- persistent SBUF ----
        sb = ctx.enter_context(tc.tile_pool(name="sb", bufs=1))
        kt_sb = sb.tile([H, S], F16, name="kt_sb")  # K^T: [h, s']
        qt_sb = sb.tile([H, R], F16, name="qt_sb")  # Q^T: [h, r]
        v_r = sb.tile([128, S], mybir.dt.float32r, name="v_r")  # V, f32r
        v_stage = ExitStack()
        v_pool = v_stage.enter_context(tc.tile_pool(name="v_pool", bufs=1))
        v_sb = v_pool.tile([128, S], F16, name="v_sb")  # chunk c at cols c*H
        ccol_sb = sb.tile([128, NT], F32, name="ccol_sb")
        ident = sb.tile([128, 128], F32, name="ident")
        d_sb = sb.tile([128, NT], F32, name="d_sb")  # rowsum(E) per row-tile
        a_sb = sb.tile([128, NT], F32, name="a_sb")  # affine mult per row-tile
        b_sb = sb.tile([128, NT], F32, name="b_sb")  # affine add per row-tile
        sumv_sb = sb.tile([1, H], F32, name="sumv_sb")
        sumvb_sb = sb.tile([128, H], F32, name="sumvb_sb")
        ones1_sb = sb.tile([1, 128], F32, name="ones1_sb")

        # qt + first kt pieces on the sync queue (gate the first matmuls);
        # the rest on the gpsimd queue so issue slots don't serialize
        nc.sync.dma_start(out=qt_sb[:, :], in_=qt[:, :])
        for p in range(3):
            cols = slice(p * (S // 4), (p + 1) * (S // 4))
            nc.sync.dma_start(out=kt_sb[:, cols], in_=kt[:, cols])
        nc.gpsimd.dma_start(out=kt_sb[:, 3 * (S // 4) :], in_=kt[:, 3 * (S // 4) :])
        nc.gpsimd.dma_start(out=v_sb[:, :], in_=vc[:, :])
        nc.gpsimd.dma_start(out=ccol_sb[:, :], in_=copy_col[:, :])
        nc.gpsimd.dma_start(out=sumv_sb[0:1, :], in_=sumv_row[0:1, :])
        # round V to float32r for the full-rate context matmuls, then drop
        # the f16 staging buffer
        for p in range(4):
            cols = slice(p * (S // 4), (p + 1) * (S // 4))
            nc.vector.tensor_copy(v_r[:, cols], v_sb[:, cols])
        v_stage.close()
        make_identity(nc, ident[:, :])

        ep = ctx.enter_context(tc.tile_pool(name="ep", bufs=2))
        ctx_acc = ep.tile([128, R], F32, name="ctx_acc", bufs=1)
        nc.vector.memset(ctx_acc[:, :], 0.0)

        with tc.tile_pool(name="b_ps", bufs=1, space="PSUM") as b_ps, \
             tc.tile_pool(name="c_ps", bufs=2, space="PSUM") as c_ps, \
             tc.tile_pool(name="b_e", bufs=3) as b_e, \
             tc.tile_pool(name="c_et", bufs=4) as c_et, \
             tc.tile_pool(name="b_small", bufs=4) as b_small:

            pending_ctx = []  # (chunk_idx, et) pairs awaiting ctx matmuls

            def flush_ctx():
                # 4 matmuls accumulate a chunk-pair's E^T @ V into a rotating
                # psum tile, then one DVE add folds it into ctx_acc (SBUF) —
                # no PSUM bank is held across the loop.
                if not pending_ctx:
                    return
                pair = pending_ctx[:]
                del pending_ctx[:]
                ps_p = c_ps.tile([128, R], F32, tag="cscore", name=f"pr{pair[0][0]}")
                n = len(pair)
                for idx, (c, et) in enumerate(pair):
                    vslice = v_r[:, c * 128 : (c + 1) * 128]
                    for j in range(2):
                        nc.tensor.matmul(
                            ps_p[:, j * 512 : (j + 1) * 512],
                            vslice,
                            et[:, j * 512 : (j + 1) * 512],
                            start=(idx == 0), stop=(idx == n - 1),
                            skip_group_check=True,
                        )
                nc.vector.tensor_add(ctx_acc[:, :], ctx_acc[:, :], ps_p[:, :])

            def c_chunk(c):
                # transposed-score chunk: scores^T -> exp (float32r out)
                ps_t = c_ps.tile([128, R], F32, tag="cscore", name=f"cs{c}")
                kslice = kt_sb[:, c * 128 : (c + 1) * 128]
                for j in range(2):
                    nc.tensor.matmul(
                        ps_t[:, j * 512 : (j + 1) * 512],
                        kslice,
                        qt_sb[:, j * 512 : (j + 1) * 512],
                        start=True, stop=True,
                    )
                et = c_et.tile([128, R], mybir.dt.float32r, tag="Et", name=f"et{c}")
                nc.scalar.activation(et[:, :], ps_t[:, :], EXP, scale=SCALE)
                pending_ctx.append((c, et))
                if len(pending_ctx) >= 2:
                    flush_ctx()

            # Interleaved main loop: per B score block (4 matmuls + one
            # FD=2048 exp), two-ish transposed chunks. Chunks are front-
            # loaded (9/9/9/9/9/9/6/4 per row-tile) so even the last
            # row-tile's exps stay covered without a bare C-C lockstep tail.
            chunks_per_tile = [2, 4, 6, 8, 10, 10, 12, 12]
            cchunk = 0
            for t in range(NT):
                budget = chunks_per_tile[t]
                e_t = b_e.tile([128, S], F32, tag="E", name=f"e{t}")
                dparts = b_small.tile([128, 4], F32, tag="dp", name=f"dp{t}")
                qslice = qt_sb[:, t * 128 : (t + 1) * 128]
                for g in range(4):
                    ps_s = b_ps.tile([128, 2048], F32, tag="bscore", name=f"bs{t}_{g}")
                    for j in range(4):
                        col = (g * 4 + j) * 512
                        nc.tensor.matmul(
                            ps_s[:, j * 512 : (j + 1) * 512],
                            qslice,
                            kt_sb[:, col : col + 512],
                            start=True, stop=True,
                        )
                    flush_ctx()  # previous pair's deferred ctx matmuls
                    nc.scalar.activation(
                        e_t[:, g * 2048 : (g + 1) * 2048],
                        ps_s[:, :],
                        EXP,
                        scale=SCALE,
                        accum_out=dparts[:, g : g + 1],
                    )
                    take = budget if g == 3 else min(budget, 2)
                    for _ in range(take):
                        c_chunk(cchunk)
                        cchunk += 1
                    budget -= take
                # per-row scalars: u = 1 + S*c ; a = 1/(D*u) ; b = c/u
                dcol = d_sb[:, t : t + 1]
                nc.vector.tensor_reduce(dcol, dparts[:, :], mybir.AxisListType.X, ADD)
                u_t = b_small.tile([128, 1], F32, tag="u", name=f"u{t}")
                ru_t = b_small.tile([128, 1], F32, tag="ru", name=f"ru{t}")
                du_t = b_small.tile([128, 1], F32, tag="du", name=f"du{t}")
                ccol = ccol_sb[:, t : t + 1]
                nc.vector.tensor_scalar(u_t[:, :], ccol, float(S), 1.0, MULT, ADD)
                nc.vector.reciprocal(ru_t[:, :], u_t[:, :])
                nc.vector.tensor_mul(du_t[:, :], dcol, u_t[:, :])
                nc.vector.reciprocal(a_sb[:, t : t + 1], du_t[:, :])
                nc.vector.tensor_mul(b_sb[:, t : t + 1], ccol, ru_t[:, :])
                # alphas tile = a*E + b, in place, then write back; split in
                # four so no single DVE op blocks the pipeline for long
                for g in range(4):
                    cols = slice(g * 2048, (g + 1) * 2048)
                    nc.vector.tensor_scalar(
                        e_t[:, cols], e_t[:, cols],
                        a_sb[:, t : t + 1], b_sb[:, t : t + 1],
                        MULT, ADD,
                    )
                nc.sync.dma_start(
                    out=alphas[t * 128 : (t + 1) * 128, :], in_=e_t[:, :]
                )

            while cchunk < NC:
                c_chunk(cchunk)
                cchunk += 1
            flush_ctx()

        # epilogue: transpose ctx^T back to row-major, scale by a, add b*colsum(v)
        with tc.tile_pool(name="ep_ps", bufs=4, space="PSUM") as ep_ps:
            # replicate colsum(v) across partitions via a K=1 fp32 matmul
            nc.vector.memset(ones1_sb[0:1, :], 1.0)
            psvb = ep_ps.tile([128, H], F32, tag="psvb", name="psvb", bufs=1)
            nc.tensor.matmul(
                psvb[:, :], ones1_sb[0:1, :], sumv_sb[0:1, :], start=True, stop=True
            )
            nc.vector.tensor_copy(sumvb_sb[:, :], psvb[:, :])
            for t in range(NT):
                ps_tr = ep_ps.tile([128, 128], F32, tag="pstr", name=f"ptr{t}")
                nc.tensor.matmul(
                    ps_tr[:, :],
                    ctx_acc[:, t * 128 : (t + 1) * 128],
                    ident[:, :],
                    is_transpose=True, start=True, stop=True,
                )
                co = ep.tile([128, H], F32, tag="co", name=f"co{t}", bufs=4)
                nc.vector.tensor_scalar(
                    co[:, :], ps_tr[:, :], a_sb[:, t : t + 1], None, MULT
                )
                # co += b * colsum(v)  (sumvb is colsum(v) replicated per partition)
                nc.vector.scalar_tensor_tensor(
                    co[:, :], sumvb_sb[:, :], b_sb[:, t : t + 1], co[:, :],
                    MULT, ADD,
                )
                # gpsimd queue: don't queue behind the big alphas DMAs
                nc.gpsimd.dma_start(
                    out=ctx_out[t * 128 : (t + 1) * 128, :], in_=co[:, :]
                )

    nc.compile()
    return nc


def kernel(embedding, copy, Wq, Wk, Wv, mask, _trace=False):
    global _CACHED_NC
    if _CACHED_NC is None:
        _CACHED_NC = build()
    nc = _CACHED_NC

    emb64 = np.asarray(embedding, dtype=np.float64)
    k_full = emb64 @ np.asarray(Wk, dtype=np.float64).T  # [S, H]
    q_full = emb64 @ np.asarray(Wq, dtype=np.float64).T
    v_full = emb64 @ np.asarray(Wv, dtype=np.float64).T
    kt_np = np.ascontiguousarray(k_full.T.astype(np.float16))  # [H, S]
    # v_sb layout: [p, c*H + h] = V[c*128 + p, h]
    vc_np = np.ascontiguousarray(
        v_full.reshape(NC, 128, H).transpose(1, 0, 2).reshape(128, S).astype(np.float16)
    )
    sumv_np = v_full.sum(axis=0).astype(np.float32)[None, :]

    in_maps = []
    for m in range(M):
        rows = slice(m * R, (m + 1) * R)
        c_m = np.asarray(copy[rows], dtype=np.float32).reshape(NT, 128)
        in_maps.append(
            {
                "kt": kt_np,
                "qt": np.ascontiguousarray(q_full[rows].T.astype(np.float16)),
                "vc": vc_np,
                "copy_col": np.ascontiguousarray(c_m.T),
                "sumv_row": sumv_np,
            }
        )

    res = run_bass_kernel_spmd(nc, in_maps, core_ids=list(range(M)), trace=_trace)
    global _LAST_RES
    _LAST_RES = res
    alphas = np.concatenate([r["alphas"] for r in res.results], axis=0)
    context = np.concatenate([r["ctx"] for r in res.results], axis=0)
    if _trace:
        return (context, alphas), res
    return context, alphas
